# revision 31
# baseline (speedup 1.0000x reference)
"""DeepseekV3 decoder layer (MLA + SwiGLU MLP), T=2048 prefill, fp32 out.

Sharding: sequence-parallel striped - core c owns token rows c::8 (256 rows);
the KV latent path over all 2048 tokens is replicated per core; outputs are
disjoint row sets concatenated on host.

v3 kernel notes:
- merged startup: kv path, token stats (square + ones-colsum matmuls), and
  q_a (computed directly transposed as qcT) share one scheduling scope.
- attention processes HEAD PAIRS: MLA shares K/V latents across heads, so
  score / o_latent / denominator matmuls stream both heads' query columns in
  a single instruction (half the matmul count at double the free dim).
  q_b is computed transposed (qbT = qbW.T @ qcT); rope runs in [dim, token]
  layout with host-permuted even/odd columns and one SBUF->SBUF
  partition-remap DMA; o_latent accumulates directly transposed
  (olT += c_hat_kt.T @ eT); softmax normalization fuses into the PSUM->SBUF
  copy via a matmul-broadcast row of 1/den.
- fine-grained causal skipping: only token columns q0(kt)=16*kt.. stream
  against key tile kt (striped row assignment makes this exact).
- MLP optionally runs fp8 (DoubleRow, 2x PE throughput): weights are
  pre-scaled into fp8 range on host, activations cast to fp8 on the fly,
  scales folded back in the silu and final residual ops.
"""

import numpy as np
import ml_dtypes

bfloat16 = ml_dtypes.bfloat16
float8_e4m3 = ml_dtypes.float8_e4m3

T = 2048
H = 2048
NH = 16
QLR = 1536
KVLR = 512
DN = 128
DR = 64
DV = 128
INTER = 10944
NCORES = 8
RPC = T // NCORES          # 256
NQT = RPC // 128           # 2
NTT = T // 128             # 16
NFC = H // 128             # 16
NRC = QLR // 128           # 12
NKV = KVLR // 128          # 4
NIT = 86
IPAD = NIT * 128
EPS = 1e-6
SCALE = (DN + DR) ** -0.5
THETA = 10000.0
QH = DN + DR               # 192

FP8_MLP = False
S_GU = 32.0                # host pre-scale on gate/up weights (fp8 range)
S_DW = 64.0                # host pre-scale on down weights

_CACHE = {}


def _build_module():
    import os
    MAXPH = int(os.environ.get("KERNEL_MAXPH", "9"))
    import concourse.bass as bass
    import concourse.tile as tile
    from concourse import bacc, mybir

    f32 = mybir.dt.float32
    bf16 = mybir.dt.bfloat16
    fp8 = mybir.dt.float8e4
    AF = mybir.ActivationFunctionType
    ALU = mybir.AluOpType
    DRow = mybir.MatmulPerfMode.DoubleRow

    nc = bacc.Bacc("TRN2", target_bir_lowering=False, debug=False,
                   enable_asserts=False, num_devices=NCORES)

    def inp(name, shape, dt):
        return nc.dram_tensor(name, list(shape), dt, kind="ExternalInput").ap()

    mlp_dt = fp8 if FP8_MLP else bf16

    # per-core inputs
    x_rows = inp("x_rows", [NQT, 128, H], bf16)
    xTc = inp("xTc", [NFC, 128, RPC], bf16)
    cosqT = inp("cosqT", [32, RPC], f32)
    sinqT = inp("sinqT", [32, RPC], f32)
    masks = inp("masks", [NTT, 128, RPC], bf16)
    # replicated inputs
    xT_blk = inp("xT_blk", [NTT, 128, NFC, 128], bf16)
    qa_blkT = inp("qa_blkT", [NRC, 128, NFC, 128], bf16)
    qb_blkT = inp("qb_blkT", [NH, 128, NRC, QH], bf16)
    kva_blk = inp("kva_blk", [NFC, 128, KVLR + DR], bf16)
    wuk = inp("wuk", [NH, 128, NKV, 128], bf16)
    wuv = inp("wuv", [NH, 128, NKV, DV], bf16)
    ow_blk = inp("ow_blk", [NH, 128, H], bf16)
    gu_blk = inp("gu_blk", [NIT // 2, 128, 2, 2, NFC, 128], mlp_dt)
    dw_blk = inp("dw_blk", [NIT // 2, 128, 2, H], mlp_dt)
    cosk = inp("cosk", [128, NTT, DR // 2], f32)
    sink = inp("sink", [128, NTT, DR // 2], f32)
    eye = inp("eye", [128, 128], bf16)
    ones = inp("ones", [128, 1], bf16)
    ones32 = inp("ones32", [1, 128], f32)
    eye32 = inp("eye32", [128, 128], f32)

    out_rows = nc.dram_tensor("out_rows", [NQT, 128, H], f32,
                              kind="ExternalOutput").ap()
    act_dram = nc.dram_tensor("act_scratch", [NIT // 2, 128, 2, RPC],
                              mlp_dt, kind="Internal").ap()

    from contextlib import ExitStack
    with tile.TileContext(nc) as tc, ExitStack() as ctx:
        persist = ctx.enter_context(tc.tile_pool(name="persist", bufs=1))

        def pt(shape, dt, tag):
            return persist.tile(list(shape), dt, tag=tag, name=tag)

        eps_sb = pt([128, 1], f32, "eps")
        nc.vector.memset(eps_sb[:], EPS)
        eye_sb = pt([128, 128], bf16, "eye")
        nc.sync.dma_start(out=eye_sb[:], in_=eye[:])
        ones_sb = pt([128, 1], bf16, "ones")
        nc.sync.dma_start(out=ones_sb[:], in_=ones[:])
        ones32_sb = pt([1, 128], f32, "ones32")
        nc.sync.dma_start(out=ones32_sb[:], in_=ones32[:])
        eye32_sb = pt([128, 128], f32, "eye32")
        nc.sync.dma_start(out=eye32_sb[:], in_=eye32[:])
        x_rows_sb = pt([128, NQT, H], bf16, "x_rows")
        for qt in range(NQT):
            nc.gpsimd.dma_start(out=x_rows_sb[:, qt, :], in_=x_rows[qt])
        cosqT_sb = pt([32, RPC], f32, "cosqT")
        nc.gpsimd.dma_start(out=cosqT_sb[:], in_=cosqT[:])
        sinqT_sb = pt([32, RPC], f32, "sinqT")
        nc.gpsimd.dma_start(out=sinqT_sb[:], in_=sinqT[:])
        masks_sb = pt([128, NTT, RPC], bf16, "masks_sb")
        for kt in range(NTT):
            nc.gpsimd.dma_start(out=masks_sb[:, kt, :], in_=masks[kt])
        cosk_sb = pt([128, NTT, DR // 2], f32, "cosk_sb")
        nc.gpsimd.dma_start(out=cosk_sb[:], in_=cosk[:])
        sink_sb = pt([128, NTT, DR // 2], f32, "sink_sb")
        nc.gpsimd.dma_start(out=sink_sb[:], in_=sink[:])

        rstd_all = pt([128, NTT], f32, "rstd_all")
        ssq_col = pt([128, NTT], f32, "ssq_col")
        ssq_kv = pt([128, NTT], f32, "ssq_kv")
        s_ck = pt([128, NTT], f32, "s_ck")
        c_hat = pt([128, NTT, KVLR], bf16, "c_hat")
        kT_lat = pt([128, NKV, T], bf16, "kT_lat")
        kT_rope = pt([64, T], bf16, "kT_rope")
        qcT = pt([128, NRC, RPC], bf16, "qcT")
        o_vT = pt([128, NH, RPC], bf16, "o_vT")
        hnT = pt([128, NFC, RPC], mlp_dt, "hnT")
        x2_sb = pt([128, NQT, H], f32, "x2_sb")

        # =========== phase A: kv path + stats + q_a (merged) ===========
        with tc.tile_pool(name="pA", bufs=3) as pA, \
             tc.tile_pool(name="pAw", bufs=NFC) as pAw, \
             tc.tile_pool(name="pAq", bufs=2) as pAq, \
             tc.tile_pool(name="pAd", bufs=2) as pAd, \
             tc.tile_pool(name="pAc", bufs=3) as pAc, \
             tc.tile_pool(name="pAs", bufs=1) as pAs, \
             tc.tile_pool(name="kv5ps", bufs=2, space="PSUM") as kv5ps, \
             tc.tile_pool(name="kv6ps", bufs=1, space="PSUM") as kv6ps, \
             tc.tile_pool(name="qcps", bufs=2, space="PSUM") as qcps, \
             tc.tile_pool(name="sqps", bufs=1, space="PSUM") as sqps, \
             tc.tile_pool(name="smps", bufs=2, space="PSUM") as smps:
            kvw = []
            for fc in range(NFC):
                w = pAw.tile([128, KVLR + DR], bf16, tag="kvw", name="kvw")
                nc.scalar.dma_start(out=w[:], in_=kva_blk[fc])
                kvw.append(w)
            xTc_sb = pAs.tile([128, NFC, RPC], bf16, name="xTc_sb")
            for fc in range(NFC):
                nc.scalar.dma_start(out=xTc_sb[:, fc, :], in_=xTc[fc])

            # rstd of this core's token rows, in row layout
            ssq_r = pAs.tile([128, NQT], f32, name="ssq_r")
            for qt in range(NQT):
                scr = pAd.tile([128, H], bf16, tag="sq", name="scrx")
                nc.vector.scalar_tensor_tensor(
                    scr[:], x_rows_sb[:, qt, :], 1.0, x_rows_sb[:, qt, :],
                    ALU.bypass, ALU.mult, accum_out=ssq_r[:, qt:qt + 1])
            ssq_row = pAs.tile([1, RPC], f32, name="ssq_row")
            for qt in range(NQT):
                rtp = smps.tile([128, 128], f32, tag="sm", name="rtp")
                nc.tensor.transpose(rtp[0:1, 0:128], ssq_r[:, qt:qt + 1],
                                    eye32_sb[:])
                nc.scalar.copy(ssq_row[0:1, qt * 128:(qt + 1) * 128],
                               rtp[0:1, 0:128])
            rstd_row = pAs.tile([1, RPC], f32, name="rstd_row")
            nc.scalar.activation(rstd_row[:], ssq_row[:], AF.Copy,
                                 bias=EPS, scale=1.0 / H)
            nc.vector.reciprocal(rstd_row[:], rstd_row[:])
            nc.scalar.activation(rstd_row[:], rstd_row[:], AF.Sqrt)

            sqcol = sqps.tile([1, RPC], f32, tag="sqcol", name="sqcol")

            def emit_qa_group(rc):
                qaw = pAq.tile([128, NFC, 128], bf16, tag="qaw", name="qaw")
                nc.scalar.dma_start(out=qaw[:], in_=qa_blkT[rc])
                qps = qcps.tile([128, RPC], f32, tag="qc", name="qc")
                for fc in range(NFC):
                    nc.tensor.matmul(qps[:], qaw[:, fc, :], xTc_sb[:, fc, :],
                                     start=(fc == 0), stop=(fc == NFC - 1))
                nc.scalar.copy(qcT[:, rc, :], qps[:])
                qsq = pAd.tile([128, RPC], bf16, tag="qsq", name="qsq")
                nc.scalar.activation(qsq[:], qcT[:, rc, :], AF.Square)
                nc.tensor.matmul(sqcol[:], ones_sb[:], qsq[:],
                                 start=(rc == 0), stop=(rc == NRC - 1))

            for tt in range(NTT):
                if 3 <= tt < 15:
                    emit_qa_group(tt - 3)
                xt = pA.tile([128, NFC, 128], bf16, tag="xt", name="xt")
                if tt == 0:
                    for qd in range(4):
                        nc.sync.dma_start(out=xt[:, 4 * qd:4 * qd + 4, :],
                                          in_=xT_blk[tt][:, 4 * qd:4 * qd + 4])
                else:
                    nc.sync.dma_start(out=xt[:], in_=xT_blk[tt])
                # kv_a matmuls first so PE never waits on the stats chain
                ps5 = kv5ps.tile([128, KVLR], f32, tag="kv5", name="kv5")
                ps6 = kv6ps.tile([128, DR], f32, tag="kv6", name="kv6")
                for fc in range(NFC):
                    nc.tensor.matmul(ps5[:], xt[:, fc, :],
                                     kvw[fc][:, 0:KVLR],
                                     start=(fc == 0), stop=(fc == NFC - 1))
                    nc.tensor.matmul(ps6[:], xt[:, fc, :],
                                     kvw[fc][:, KVLR:KVLR + DR],
                                     start=(fc == 0), stop=(fc == NFC - 1))
                # per-token ssq over H: square on DVE/gpsimd, ones-colsum
                sq = pAd.tile([128, NFC, 128], bf16, tag="sq", name="sq")
                nc.vector.scalar_tensor_tensor(
                    sq[:], xt[:], 1.0, xt[:], ALU.bypass, ALU.mult)
                srow = smps.tile([1, 128], f32, tag="sm", name="srow")
                for fc in range(NFC):
                    nc.tensor.matmul(srow[:], ones_sb[:], sq[:, fc, :],
                                     start=(fc == 0), stop=(fc == NFC - 1))
                srow_sb = pAd.tile([1, 128], f32, tag="srow_sb",
                                   name="srow_sb")
                nc.scalar.copy(srow_sb[:], srow[:])
                stp = smps.tile([128, 128], f32, tag="sm", name="stp")
                nc.tensor.transpose(stp[0:128, 0:1], srow_sb[:],
                                    eye32_sb[0:1, 0:1])
                nc.vector.tensor_copy(ssq_col[:, tt:tt + 1], stp[0:128, 0:1])
                rt = pAd.tile([128, 1], f32, tag="rt", name="rt")
                nc.scalar.activation(rt[:], ssq_col[:, tt:tt + 1], AF.Copy,
                                     bias=EPS, scale=1.0 / H)
                nc.vector.reciprocal(rt[:], rt[:])
                nc.scalar.activation(rstd_all[:, tt:tt + 1], rt[:], AF.Sqrt)

                c_raw = pAc.tile([128, KVLR + DR], bf16, tag="craw",
                                 name="craw")
                nc.scalar.copy(c_raw[:, 0:KVLR], ps5[:])
                nc.vector.tensor_copy(c_raw[:, KVLR:KVLR + DR], ps6[:])
                scr2 = pAd.tile([128, KVLR], bf16, tag="scr2", name="scr2")
                nc.scalar.activation(scr2[:], c_raw[:, 0:KVLR], AF.Square,
                                     accum_out=ssq_kv[:, tt:tt + 1])
                t1 = pAd.tile([128, 1], f32, tag="t1", name="t1")
                nc.vector.tensor_mul(t1[:], rstd_all[:, tt:tt + 1],
                                     rstd_all[:, tt:tt + 1])
                nc.vector.tensor_mul(t1[:], t1[:], ssq_kv[:, tt:tt + 1])
                nc.scalar.activation(t1[:], t1[:], AF.Copy,
                                     bias=EPS, scale=1.0 / KVLR)
                nc.vector.reciprocal(t1[:], t1[:])
                nc.scalar.activation(t1[:], t1[:], AF.Sqrt)
                nc.vector.tensor_mul(s_ck[:, tt:tt + 1],
                                     rstd_all[:, tt:tt + 1], t1[:])
                nc.vector.tensor_scalar_mul(c_hat[:, tt, :],
                                            c_raw[:, 0:KVLR],
                                            s_ck[:, tt:tt + 1])

                # k rope (outputs permuted: evens in cols 0:32, odds 32:64)
                kr = pAd.tile([128, DR], bf16, tag="kr", name="kr")
                nc.vector.tensor_scalar_mul(kr[:], c_raw[:, KVLR:KVLR + DR],
                                            rstd_all[:, tt:tt + 1])
                x1 = kr[:, 0:DR:2]
                x2 = kr[:, 1:DR:2]
                ta = pAd.tile([128, DR // 2], f32, tag="ta", name="ta")
                tb = pAd.tile([128, DR // 2], f32, tag="tb", name="tb")
                krf = pAd.tile([128, DR], bf16, tag="krf", name="krf")
                nc.vector.tensor_mul(ta[:], x1, cosk_sb[:, tt, :])
                nc.vector.tensor_mul(tb[:], x2, sink_sb[:, tt, :])
                nc.vector.tensor_sub(krf[:, 0:32], ta[:], tb[:])
                nc.vector.tensor_mul(ta[:], x2, cosk_sb[:, tt, :])
                nc.vector.tensor_mul(tb[:], x1, sink_sb[:, tt, :])
                nc.vector.tensor_add(krf[:, 32:64], ta[:], tb[:])

                for rc in range(NKV):
                    tp = smps.tile([128, 128], bf16, tag="sm", name="tp")
                    nc.tensor.transpose(
                        tp[:], c_hat[:, tt, rc * 128:(rc + 1) * 128],
                        eye_sb[:])
                    nc.scalar.copy(kT_lat[:, rc, tt * 128:(tt + 1) * 128],
                                   tp[:])
                tpr = smps.tile([128, 128], bf16, tag="sm", name="tpr")
                nc.tensor.transpose(tpr[0:64, :], krf[:], eye_sb[:])
                nc.scalar.copy(kT_rope[:, tt * 128:(tt + 1) * 128],
                               tpr[0:64, :])

            s2 = pAs.tile([1, RPC], f32, name="s2")
            nc.vector.tensor_mul(s2[:], rstd_row[:], rstd_row[:])
            nc.vector.tensor_mul(s2[:], s2[:], sqcol[:])
            nc.scalar.activation(s2[:], s2[:], AF.Copy,
                                 bias=EPS, scale=1.0 / QLR)
            nc.vector.reciprocal(s2[:], s2[:])
            nc.scalar.activation(s2[:], s2[:], AF.Sqrt)
            sq_row = pAs.tile([1, RPC], f32, name="sq_row")
            nc.vector.tensor_mul(sq_row[:], rstd_row[:], s2[:])
            sqb_ps = qcps.tile([128, RPC], f32, tag="qc", name="sqb_ps")
            nc.tensor.matmul(sqb_ps[:], ones32_sb[:], sq_row[:],
                             start=True, stop=True)
            sqb = pt([128, RPC], f32, "sqb")
            nc.vector.tensor_copy(sqb[:], sqb_ps[:])

        # =================== phase B: attention (head pairs) ==============
        # oww prefetch pool opened early so o_proj weights stream during B
        ctxBC = ExitStack()
        pC = ctxBC.enter_context(tc.tile_pool(name="pC", bufs=4))
        if MAXPH >= 2:
            with tc.tile_pool(name="pB", bufs=2) as pB, \
                 tc.tile_pool(name="pBd", bufs=2) as pBd, \
                 tc.tile_pool(name="pBe", bufs=4) as pBe, \
                 tc.tile_pool(name="qbps", bufs=1, space="PSUM") as qbps, \
                 tc.tile_pool(name="spps", bufs=2, space="PSUM") as spps, \
                 tc.tile_pool(name="olps", bufs=1, space="PSUM") as olps, \
                 tc.tile_pool(name="denps", bufs=1, space="PSUM") as denps:
                for hp in range(NH // 2):
                    heads = (2 * hp, 2 * hp + 1)
                    qT = pBd.tile([128, 5, 2, RPC], bf16, tag="qT", name="qT")
                    wuv_p = []
                    for i, h in enumerate(heads):
                        qbw = pB.tile([128, NRC, QH], bf16, tag="qbw",
                                      name="qbw")
                        nc.sync.dma_start(out=qbw[:], in_=qb_blkT[h])
                        wuk_h = pB.tile([128, NKV, 128], bf16, tag="wuk",
                                        name="wuk_h")
                        nc.sync.dma_start(out=wuk_h[:], in_=wuk[h])
                        wuv_h = pB.tile([128, NKV, DV], bf16, tag="wuv",
                                        name="wuv_h")
                        nc.sync.dma_start(out=wuv_h[:], in_=wuv[h])
                        wuv_p.append(wuv_h)

                        # q_b transposed: nope chain then rope chain
                        # (sequential through one PSUM slot: start=True
                        # zeroes the whole bank, so the chains cannot share)
                        qbt = qbps.tile([128, RPC], f32, tag="qbt",
                                        name="qbt")
                        for rc in range(NRC):
                            nc.tensor.matmul(qbt[:], qbw[:, rc, 0:128],
                                             qcT[:, rc, :], start=(rc == 0),
                                             stop=(rc == NRC - 1))
                        qnT = pBd.tile([128, RPC], bf16, tag="qnT",
                                       name="qnT")
                        nc.vector.tensor_mul(qnT[:], qbt[:], sqb[:])
                        qbt2 = qbps.tile([128, RPC], f32, tag="qbt",
                                         name="qbt2")
                        for rc in range(NRC):
                            nc.tensor.matmul(qbt2[0:64, :],
                                             qbw[:, rc, 128:192],
                                             qcT[:, rc, :], start=(rc == 0),
                                             stop=(rc == NRC - 1))
                        ropeA = pBd.tile([64, RPC], f32, tag="ropeA",
                                         name="ropeA")
                        nc.vector.tensor_mul(ropeA[:], qbt2[0:64, :],
                                             sqb[0:64, :])
                        ropeB = pBd.tile([32, RPC], f32, tag="ropeB",
                                         name="ropeB")
                        nc.scalar.dma_start(out=ropeB[:], in_=ropeA[32:64, :])
                        ta = pBd.tile([32, RPC], f32, tag="ta", name="taq")
                        tb = pBd.tile([32, RPC], f32, tag="tb", name="tbq")
                        nc.vector.tensor_mul(ta[:], ropeA[0:32, :],
                                             cosqT_sb[:])
                        nc.vector.tensor_mul(tb[:], ropeB[:], sinqT_sb[:])
                        nc.vector.tensor_sub(qT[0:32, 4, i, :], ta[:], tb[:])
                        nc.vector.tensor_mul(ta[:], ropeB[:], cosqT_sb[:])
                        nc.vector.tensor_mul(tb[:], ropeA[0:32, :],
                                             sinqT_sb[:])
                        r2t = pBd.tile([32, RPC], bf16, tag="r2t", name="r2t")
                        nc.vector.tensor_add(r2t[:], ta[:], tb[:])
                        nc.scalar.dma_start(out=qT[32:64, 4, i, :],
                                            in_=r2t[:])

                        # absorb: latent q chunks = wuk.T @ qnT
                        for rc in range(NKV):
                            lp = spps.tile([128, 2, RPC], f32, tag="sp",
                                           name="lp")
                            nc.tensor.matmul(lp[:, 0, :], wuk_h[:, rc, :],
                                             qnT[:], start=True, stop=True)
                            nc.scalar.copy(qT[:, rc, i, :], lp[:, 0, :])

                    olt = [olps.tile([128, 2, RPC], f32, tag=f"ol{b}",
                                     name=f"ol{b}") for b in range(NKV)]
                    den_t = denps.tile([128, 2, RPC], f32, tag="den",
                                       name="den")
                    den = den_t[0:1]
                    for kt in range(NTT):
                        q0 = 16 * kt if kt < 8 else 128
                        sp = spps.tile([128, 2, RPC], f32, tag="sp",
                                       name="sp")
                        for dc in range(NKV):
                            nc.tensor.matmul(
                                sp[:, :, q0:RPC],
                                kT_lat[:, dc, kt * 128:(kt + 1) * 128],
                                qT[:, dc, :, q0:RPC],
                                start=(dc == 0), stop=False)
                        nc.tensor.matmul(
                            sp[:, :, q0:RPC],
                            kT_rope[:, kt * 128:(kt + 1) * 128],
                            qT[0:64, 4, :, q0:RPC],
                            start=False, stop=True)
                        eT = pBe.tile([128, 2, RPC], bf16, tag="eT",
                                      name="eT")
                        nc.scalar.activation(eT[:, :, q0:RPC],
                                             sp[:, :, q0:RPC], AF.Exp)
                        for i in range(2):
                            nc.vector.tensor_mul(eT[:, i, q0:RPC],
                                                 eT[:, i, q0:RPC],
                                                 masks_sb[:, kt, q0:RPC])
                        for rc in range(NKV):
                            nc.tensor.matmul(
                                olt[rc][:, :, q0:RPC],
                                c_hat[:, kt, rc * 128:(rc + 1) * 128],
                                eT[:, :, q0:RPC],
                                start=(kt == 0), stop=(kt == NTT - 1),
                                skip_group_check=True)
                        nc.tensor.matmul(den[0:1, :, q0:RPC],
                                         ones_sb[:], eT[:, :, q0:RPC],
                                         start=(kt == 0), stop=(kt == NTT - 1),
                                         skip_group_check=True)

                    denr = pBd.tile([1, 2 * RPC], f32, tag="denr",
                                    name="denr")
                    nc.vector.reciprocal(denr[:], den[0:1, :, :])
                    rb = denps.tile([128, 2, RPC], f32, tag="den",
                                    name="rb")
                    nc.tensor.matmul(rb[:, :, :], ones32_sb[:],
                                     denr[:], start=True, stop=True)
                    rbs = pBd.tile([128, 2, RPC], f32, tag="rbs", name="rbs")
                    nc.scalar.copy(rbs[:], rb[:])
                    oln = pBd.tile([128, NKV, 2, RPC], bf16, tag="oln",
                                   name="oln")
                    for rc in range(NKV):
                        for i in range(2):
                            nc.vector.tensor_mul(
                                oln[:, rc, i, :], olt[rc][:, i, :],
                                rbs[:, i, :])
                    for i, h in enumerate(heads):
                        ovp = denps.tile([128, 2, RPC], f32, tag="den",
                                         name="ovp")
                        for rc in range(NKV):
                            nc.tensor.matmul(ovp[:, 0, :],
                                             wuv_p[i][:, rc, :],
                                             oln[:, rc, i, :],
                                             start=(rc == 0),
                                             stop=(rc == NKV - 1))
                        nc.scalar.copy(o_vT[:, h, :], ovp[:, 0, :])

        # ============ phase C: o_proj + residual + post-norm ============
        if MAXPH >= 3:
            with tc.tile_pool(name="pCs", bufs=1) as pCs, \
                 tc.tile_pool(name="pCd", bufs=2) as pCd:
                hn = pCs.tile([128, NQT, H], bf16, name="hn")
                with tc.tile_pool(name="opps", bufs=1, space="PSUM") as opps:
                    op = [opps.tile([128, H], f32, tag=f"op{qt}",
                                    name=f"op{qt}") for qt in range(NQT)]
                    for hc in range(NH):
                        oww = pC.tile([128, H], bf16, tag="oww", name="oww")
                        nc.sync.dma_start(out=oww[:], in_=ow_blk[hc])
                        for qt in range(NQT):
                            for nn in range(4):
                                nc.tensor.matmul(
                                    op[qt][:, nn * 512:(nn + 1) * 512],
                                    o_vT[:, hc, qt * 128:(qt + 1) * 128],
                                    oww[:, nn * 512:(nn + 1) * 512],
                                    start=(hc == 0), stop=(hc == NH - 1))
                    ssq2 = pCs.tile([128, NQT], f32, name="ssq2")
                    for qt in range(NQT):
                        nc.vector.tensor_add(x2_sb[:, qt, :],
                                             x_rows_sb[:, qt, :], op[qt][:])
                    scrA = pCd.tile([128, H], bf16, tag="scr3",
                                    name="scrA")
                    nc.scalar.activation(scrA[:], x2_sb[:, 0, :], AF.Square,
                                         accum_out=ssq2[:, 0:1])
                    scr = pCd.tile([128, H], bf16, tag="scr3", name="scr3")
                    nc.vector.scalar_tensor_tensor(
                        scr[:], x2_sb[:, 1, :], 1.0, x2_sb[:, 1, :],
                        ALU.bypass, ALU.mult, accum_out=ssq2[:, 1:2])
                    nc.scalar.activation(ssq2[:], ssq2[:], AF.Copy,
                                         bias=EPS, scale=1.0 / H)
                    nc.vector.reciprocal(ssq2[:], ssq2[:])
                    nc.scalar.activation(ssq2[:], ssq2[:], AF.Sqrt)
                    nc.scalar.activation(hn[:, 0, :], x2_sb[:, 0, :],
                                         AF.Copy, scale=ssq2[:, 0:1])
                    nc.vector.tensor_scalar_mul(hn[:, 1, :],
                                                x2_sb[:, 1, :],
                                                ssq2[:, 1:2])
                with tc.tile_pool(name="tpps", bufs=2, space="PSUM") as tpps:
                    for qt in range(NQT):
                        for fc in range(NFC):
                            tp = tpps.tile([128, 128], bf16, tag="tp",
                                           name="tp")
                            nc.tensor.transpose(
                                tp[:], hn[:, qt, fc * 128:(fc + 1) * 128],
                                eye_sb[:])
                            nc.scalar.copy(
                                hnT[:, fc, qt * 128:(qt + 1) * 128], tp[:])

        ctxBC.close()

        # =================== phase D: MLP ===================
        if MAXPH >= 4:
            with tc.tile_pool(name="pD", bufs=3) as pD, \
                 tc.tile_pool(name="pDw", bufs=3) as pDw:
                with tc.tile_pool(name="pDps", bufs=3, space="PSUM") as pDps:
                    for it in range(NIT):
                        if it % 2 == 0:
                            guw2 = pD.tile([128, 2, 2, NFC, 128], mlp_dt,
                                           tag="guw", name="guw")
                            nc.sync.dma_start(out=guw2[:],
                                              in_=gu_blk[it // 2])
                        guw = guw2[:, it % 2]
                        gp = pDps.tile([128, RPC], f32, tag="gp", name="gp")
                        up = pDps.tile([128, RPC], f32, tag="up", name="up")
                        if FP8_MLP:
                            for j in range(NFC // 2):
                                nc.tensor.matmul(
                                    gp[:], guw[:, 0, 2 * j:2 * j + 2, :],
                                    hnT[:, 2 * j:2 * j + 2, :],
                                    start=(j == 0), stop=(j == NFC // 2 - 1),
                                    perf_mode=DRow)
                                nc.tensor.matmul(
                                    up[:], guw[:, 1, 2 * j:2 * j + 2, :],
                                    hnT[:, 2 * j:2 * j + 2, :],
                                    start=(j == 0), stop=(j == NFC // 2 - 1),
                                    perf_mode=DRow)
                        else:
                            for fc in range(NFC):
                                nc.tensor.matmul(gp[:], guw[:, 0, fc, :],
                                                 hnT[:, fc, :],
                                                 start=(fc == 0),
                                                 stop=(fc == NFC - 1))
                                nc.tensor.matmul(up[:], guw[:, 1, fc, :],
                                                 hnT[:, fc, :],
                                                 start=(fc == 0),
                                                 stop=(fc == NFC - 1))
                        gs = pD.tile([128, RPC], bf16, tag="gs", name="gs")
                        # psum holds S_GU*g: silu(g) via LUT input scale
                        nc.scalar.activation(gs[:], gp[:], AF.Silu,
                                             scale=(1.0 / S_GU
                                                    if FP8_MLP else 1.0))
                        # act = silu(g) * (up_psum / S_GU)
                        act_t = pD.tile([128, RPC], mlp_dt, tag="act",
                                        name="act_t")
                        nc.vector.scalar_tensor_tensor(
                            act_t[:], gs[:],
                            (1.0 / S_GU if FP8_MLP else 1.0),
                            up[:], ALU.mult, ALU.mult)
                        nc.gpsimd.dma_start(
                            out=act_dram[it // 2][:, it % 2, :],
                            in_=act_t[:])
                with tc.tile_pool(name="oDps", bufs=1, space="PSUM") as oDps:
                    o_ps = [oDps.tile([128, H], f32, tag=f"ops{qt}",
                                      name=f"ops{qt}") for qt in range(NQT)]
                    if FP8_MLP:
                        for ip in range(NIT // 2):
                            dw = pDw.tile([128, 2, H], fp8, tag="dw",
                                          name="dw")
                            nc.scalar.dma_start(out=dw[:],
                                                in_=dw_blk[ip])
                            act_rd = pDw.tile([128, 2, RPC], fp8,
                                              tag="actr", name="act_rd")
                            nc.sync.dma_start(
                                out=act_rd[:],
                                in_=act_dram[2 * ip:2 * ip + 2])
                            for qt in range(NQT):
                                for nn in range(4):
                                    nc.tensor.matmul(
                                        o_ps[qt][:, nn * 512:(nn + 1) * 512],
                                        act_rd[:, :,
                                               qt * 128:(qt + 1) * 128],
                                        dw[:, :, nn * 512:(nn + 1) * 512],
                                        start=(ip == 0),
                                        stop=(ip == NIT // 2 - 1),
                                        perf_mode=DRow)
                    else:
                        for it in range(NIT):
                            if it % 2 == 0:
                                dw2 = pDw.tile([128, 2, H], bf16, tag="dw",
                                               name="dw")
                                nc.scalar.dma_start(out=dw2[:],
                                                    in_=dw_blk[it // 2])
                                act2 = pDw.tile([128, 2, RPC], bf16,
                                                tag="actr", name="act_rd")
                                nc.gpsimd.dma_start(out=act2[:],
                                                    in_=act_dram[it // 2])
                            dw = dw2[:, it % 2]
                            act_rd = act2[:, it % 2]
                            for qt in range(NQT):
                                for nn in range(4):
                                    nc.tensor.matmul(
                                        o_ps[qt][:, nn * 512:(nn + 1) * 512],
                                        act_rd[:,
                                               qt * 128:(qt + 1) * 128],
                                        dw[:, nn * 512:(nn + 1) * 512],
                                        start=(it == 0), stop=(it == NIT - 1))
                    for qt in range(NQT):
                        fin = pD.tile([128, H], f32, tag="fin", name="fin")
                        for ch in range(4):
                            cs = slice(ch * 512, (ch + 1) * 512)
                            nc.vector.scalar_tensor_tensor(
                                fin[:, cs], o_ps[qt][:, cs],
                                (1.0 / S_DW if FP8_MLP else 1.0),
                                x2_sb[:, qt, cs], ALU.mult, ALU.add)
                            nc.sync.dma_start(out=out_rows[qt][:, cs],
                                              in_=fin[:, cs])

        if MAXPH < 4:
            with tc.tile_pool(name="pex", bufs=2) as pex:
                for qt in range(NQT):
                    fin = pex.tile([128, H], f32, tag="finx", name="finx")
                    nc.vector.tensor_copy(fin[:], x2_sb[:, qt, :]
                                          if MAXPH >= 3 else
                                          x_rows_sb[:, qt, :])
                    nc.sync.dma_start(out=out_rows[qt], in_=fin[:])
    nc.compile()
    return nc


def _host_prep(inputs):
    f32 = np.float32
    bf = bfloat16
    x = np.asarray(inputs["hidden_states"], f32)
    pos = np.asarray(inputs["positions"]).astype(f32)

    lnw_in = np.asarray(inputs["input_ln_w"], f32)
    q_a_w = np.asarray(inputs["q_a_w"], f32) * lnw_in[:, None]
    kv_a_w = np.asarray(inputs["kv_a_w"], f32) * lnw_in[:, None]
    q_b_w = (np.asarray(inputs["q_b_w"], f32)
             * np.asarray(inputs["q_a_ln_w"], f32)[:, None]) * SCALE
    kvln = np.asarray(inputs["kv_a_ln_w"], f32)
    w_uk = np.asarray(inputs["w_uk"], f32) * kvln[:, None, None]
    w_uv = np.asarray(inputs["w_uv"], f32) * kvln[:, None, None]
    o_w = np.asarray(inputs["o_w"], f32)
    pln = np.asarray(inputs["post_ln_w"], f32)
    gate_w = np.asarray(inputs["gate_w"], f32) * pln[:, None]
    up_w = np.asarray(inputs["up_w"], f32) * pln[:, None]
    down_w = np.asarray(inputs["down_w"], f32)

    # permute each head's rope columns of q_b: evens first, then odds
    perm = np.concatenate([np.arange(0, DR, 2), np.arange(1, DR, 2)])
    qb = q_b_w.reshape(QLR, NH, QH)
    qb_rope = qb[:, :, DN:][:, :, perm]
    qb = np.concatenate([qb[:, :, :DN], qb_rope], axis=2)  # [QLR, NH, QH]

    xT = np.ascontiguousarray(x.T)
    inv_freq = 1.0 / (THETA ** (np.arange(0, DR, 2, dtype=f32) / DR))
    ang = pos[:, None] * inv_freq
    cos_t = np.cos(ang).astype(f32)
    sin_t = np.sin(ang).astype(f32)

    gu = np.zeros((2, IPAD, H), f32)
    gu[0, :INTER] = gate_w.T
    gu[1, :INTER] = up_w.T
    dwp = np.concatenate([down_w, np.zeros((IPAD - INTER, H), f32)], 0)
    if FP8_MLP:
        mdt = float8_e4m3
        gu = gu * S_GU
        dwp = dwp * S_DW
    else:
        mdt = bf

    rep = {
        "xT_blk": np.ascontiguousarray(
            xT.astype(bf).reshape(NFC, 128, NTT, 128).transpose(2, 1, 0, 3)),
        # qa_blkT[rc, p, fc, m] = q_a_w[fc*128+p, rc*128+m]
        "qa_blkT": np.ascontiguousarray(
            q_a_w.astype(bf).reshape(NFC, 128, NRC, 128)
            .transpose(2, 1, 0, 3)),
        # qb_blkT[h, p, rc, d] = qb[rc*128+p, h, d]
        "qb_blkT": np.ascontiguousarray(
            qb.astype(bf).reshape(NRC, 128, NH, QH).transpose(2, 1, 0, 3)),
        "kva_blk": np.ascontiguousarray(
            kv_a_w.astype(bf).reshape(NFC, 128, KVLR + DR)),
        # wuk[h, d, rc, rr] = w_uk[rc*128+rr, h, d]
        "wuk": np.ascontiguousarray(
            w_uk.transpose(1, 2, 0).reshape(NH, 128, NKV, 128).astype(bf)),
        # wuv[h, p, rc, dv] = w_uv[rc*128+p, h, dv]
        "wuv": np.ascontiguousarray(
            w_uv.transpose(1, 0, 2).reshape(NH, NKV, 128, DV)
            .transpose(0, 2, 1, 3).astype(bf)),
        "ow_blk": np.ascontiguousarray(o_w.astype(bf).reshape(NH, 128, H)),
        # gu_blk[ip, p, s, g, fc, c] = gu[g, (2*ip+s)*128+c, fc*128+p]
        "gu_blk": np.ascontiguousarray(
            gu.reshape(2, NIT // 2, 2, 128, NFC, 128)
            .transpose(1, 5, 2, 0, 4, 3).astype(mdt)),
        # dw_blk[ip, p, s, col] = down_w[(2*ip+s)*128+p, col]
        "dw_blk": np.ascontiguousarray(
            dwp.astype(mdt).reshape(NIT // 2, 2, 128, H)
            .transpose(0, 2, 1, 3)),
        "cosk": np.ascontiguousarray(
            cos_t.reshape(NTT, 128, DR // 2).transpose(1, 0, 2)),
        "sink": np.ascontiguousarray(
            sin_t.reshape(NTT, 128, DR // 2).transpose(1, 0, 2)),
        "eye": np.eye(128, dtype=bf),
        "ones": np.ones((128, 1), bf),
        "ones32": np.ones((1, 128), f32),
        "eye32": np.eye(128, dtype=f32),
    }

    per_core = []
    for c in range(NCORES):
        rows = np.arange(c, T, NCORES)
        m = dict(rep)
        m["x_rows"] = np.ascontiguousarray(
            x[rows].reshape(NQT, 128, H).astype(bf))
        m["xTc"] = np.ascontiguousarray(
            xT[:, rows].astype(bf).reshape(NFC, 128, RPC))
        m["cosqT"] = np.ascontiguousarray(cos_t[rows].T)      # [32, RPC]
        m["sinqT"] = np.ascontiguousarray(sin_t[rows].T)
        mask = np.zeros((NTT, 128, RPC), f32)
        kpos = np.arange(128)
        for kt in range(NTT):
            gk = kt * 128 + kpos
            mask[kt] = (gk[:, None] <= rows[None, :]).astype(f32)
        m["masks"] = mask.astype(bf)
        per_core.append(m)
    return per_core


def kernel(**inputs):
    from concourse import bass_utils

    if "nc" not in _CACHE:
        _CACHE["nc"] = _build_module()
    nc = _CACHE["nc"]

    import os
    in_maps = _host_prep(inputs)
    trace = bool(os.environ.get("BASS_KERNEL_TRACE"))
    res = bass_utils.run_bass_kernel_spmd(nc, in_maps,
                                          core_ids=list(range(NCORES)),
                                          trace=trace)
    if trace and res.exec_time_ns is not None:
        print(f"HW exec time: {res.exec_time_ns} ns")
        _CACHE["last_result"] = res
    out = np.zeros((T, H), np.float32)
    for c in range(NCORES):
        rows = np.arange(c, T, NCORES)
        out[rows] = res.results[c]["out_rows"].reshape(RPC, H)
    return out
